# revision 13
# baseline (speedup 1.0000x reference)
"""Mesh vertex-normals kernel for 8 trn2 NeuronCores (Bass/Tile).

The reference problem: area-weighted vertex normals of a structured
GRID x GRID triangulated mesh (every quad -> 2 triangles), faces given as
an explicit [n_faces, 3] int32 array.

Key algebraic facts exploited here:
  * weighted face normal = unit_normal * area = cross(vb-va, vc-vb) * 0.5
    (the normalization by |cross| cancels against the area factor), and the
    final per-vertex normalization makes even the 0.5 factor irrelevant.
  * for the structured triangulation produced by setup_inputs(), the
    scatter-add over faces collapses into a fixed 2x2 stencil over the
    vertex grid -> no scatter, no gather, no collectives are needed.
    Each core processes a horizontal band of the grid with 1-row halos.
  * edge-duplicated padding (np.pad mode='edge') makes all phantom quads
    outside the grid degenerate (zero cross product), so boundary vertices
    need no special-casing on device.

Per-core device program (all f32, AoS [row, col, xyz] layout, grid rows on
partitions). Engines cannot read an SBUF operand at a nonzero partition
offset, so the two row-neighbor relations are realized as:
  * V / Vdn: the vertex band loaded twice from DRAM, offset by one row.
  * S[s] = P[s+1] + Q[s]: the P row-shift runs on the (otherwise idle)
    tensor engine as a shifted-identity matmul into PSUM.

    HX  = V[:,c+1]-V[:,c]         (row-aligned horizontal edges)
    HXd = Vdn[:,c+1]-Vdn[:,c]     (row+1 horizontal edges)
    VY  = Vdn-V                   (vertical edges)
    d   = HX + VY[:,c+1]          (quad diagonal)
    C1  = cross(HX, VY[:,c+1])    (triangle 1 weighted normal x2)
    C2  = cross(HXd, d)           (triangle 2 weighted normal x2)
    T   = C1 + C2
    P   = T[:,c+1] + C1           (terms needing quad row s+1)
    Q   = T + C2[:,c+1]           (terms needing quad row s)
    S   = shift_down(P) + Q       (shift via PE matmul)
    out = S * (1 / sqrt(sum(S^2) + tiny))

If `faces` does not match the structured triangulation (it always does for
the reference setup_inputs), we fall back to an exact host computation.
"""

import sys

sys.path.insert(0, "/opt/trn_rl_repo")

import numpy as np

GRID = 1449
N_CORES = 8
COL_CHUNK = 510   # output columns per on-chip tile (3*510 f32 per partition)
PSUM_COLS = 170   # columns per PSUM bank chunk (3*170 = 510 f32 <= 512)


# ---------------------------------------------------------------------------
# host-side helpers
# ---------------------------------------------------------------------------

def _is_structured(faces: np.ndarray, grid: int) -> bool:
    n_quads = (grid - 1) * (grid - 1)
    if faces.shape != (2 * n_quads, 3):
        return False
    idx = np.arange(grid * grid, dtype=np.int64).reshape(grid, grid)
    i00 = idx[:-1, :-1].ravel()
    i01 = idx[:-1, 1:].ravel()
    i10 = idx[1:, :-1].ravel()
    i11 = idx[1:, 1:].ravel()
    f = faces
    return (
        np.array_equal(f[:n_quads, 0], i00)
        and np.array_equal(f[:n_quads, 1], i01)
        and np.array_equal(f[:n_quads, 2], i11)
        and np.array_equal(f[n_quads:, 0], i00)
        and np.array_equal(f[n_quads:, 1], i11)
        and np.array_equal(f[n_quads:, 2], i10)
    )


def _host_fallback(vertices: np.ndarray, faces: np.ndarray) -> np.ndarray:
    """Exact replica of the reference for non-structured faces."""
    n_vertices = vertices.shape[0]
    va = vertices[faces[:, 0]]
    vb = vertices[faces[:, 1]]
    vc = vertices[faces[:, 2]]
    cross = np.cross(vb - va, vc - vb).astype(np.float32)
    norm = np.linalg.norm(cross, axis=-1, keepdims=True)
    weighted = (cross / norm) * (norm * 0.5)
    data = np.broadcast_to(weighted[:, None, :], (faces.shape[0], 3, 3)).reshape(-1, 3)
    summed = np.zeros((n_vertices, 3), dtype=np.float32)
    np.add.at(summed, faces.reshape(-1), data)
    norms = np.linalg.norm(summed, axis=-1, keepdims=True)
    return (summed / np.maximum(norms, 1e-10)).astype(np.float32)


def _band_layout(grid: int, n_cores: int):
    """Row-band sharding: core k outputs rows [base*k, base*k + base + 1)."""
    base = (grid - 1) // n_cores
    assert base * n_cores == grid - 1, "grid-1 must divide evenly"
    out_rows = base + 1          # per-core output rows (1-row overlap)
    in_rows = base + 3           # with halo rows (padded coords)
    return base, out_rows, in_rows


def _row_blocks(n_v_rows: int):
    """Split a band's V rows into <=128-partition blocks, overlapping by 2."""
    blocks = []
    r0 = 0
    while r0 < n_v_rows - 2:
        nv = min(128, n_v_rows - r0)
        blocks.append((r0, nv))
        r0 += nv - 2
    return blocks


def _col_chunks(width: int, chunk: int):
    return [(c0, min(chunk, width - c0)) for c0 in range(0, width, chunk)]


# ---------------------------------------------------------------------------
# device program
# ---------------------------------------------------------------------------

def _build_program(grid: int, n_cores: int, repeats: int = 1):
    import contextlib

    import concourse.bacc as bacc
    import concourse.tile as tile
    from concourse import mybir
    from concourse.masks import make_identity

    f32 = mybir.dt.float32
    Alu = mybir.AluOpType
    Act = mybir.ActivationFunctionType

    base, out_rows, in_rows = _band_layout(grid, n_cores)
    W = grid + 2      # padded columns

    nc = bacc.Bacc()
    vband = nc.dram_tensor("vband", [in_rows, W, 3], f32, kind="ExternalInput")
    oband = nc.dram_tensor("oband", [out_rows, grid, 3], f32, kind="ExternalOutput")

    with tile.TileContext(nc) as tc:
        with (
            tc.tile_pool(name="io", bufs=3) as io,
            tc.tile_pool(name="wk", bufs=1) as wk,
            tc.tile_pool(name="ps", bufs=4, space="PSUM") as psp,
            tc.tile_pool(name="cst", bufs=1) as cst,
        ):
            eps_tile = cst.tile([128, 1], f32, tag="eps")
            nc.vector.memset(eps_tile[:, :], 1e-30)
            # tid[:, :128] = I; columns 128/129 zero => tid[:, 1:129] is the
            # down-shift matrix SH[k, m] = 1 iff k == m+1.
            tid = cst.tile([128, 130], f32, tag="tid")
            nc.gpsimd.memset(tid[:, :], 0.0)
            make_identity(nc, tid[:, 0:128], nomemset=True)

            loop = tc.For_i(0, repeats, 1) if repeats > 1 else contextlib.nullcontext()
            with loop:
                _emit_body(nc, tc, io, wk, psp, eps_tile, tid,
                           vband, oband, grid, in_rows, Alu, Act, f32)

    nc.finalize()
    return nc


def _emit_body(nc, tc, io, wk, psp, eps_tile, tid, vband, oband,
               grid, in_rows, Alu, Act, f32):
    W = grid + 2
    if True:  # keep the original indentation structure
            for r0, nv in _row_blocks(in_rows):
                nq = nv - 1   # quad rows in this block
                ns = nv - 2   # output rows in this block
                for c0, w in _col_chunks(grid, COL_CHUNK):
                    # loads (second one shifted down a row)
                    v = io.tile([nv, w + 2, 3], f32, tag="v")
                    nc.sync.dma_start(
                        out=v[:, :, :],
                        in_=vband[r0 : r0 + nv, c0 : c0 + w + 2, :],
                    )
                    vd = io.tile([nq, w + 2, 3], f32, tag="vd")
                    nc.sync.dma_start(
                        out=vd[:, :, :],
                        in_=vband[r0 + 1 : r0 + nv, c0 : c0 + w + 2, :],
                    )

                    # edge fields
                    hx = wk.tile([nq, w + 1, 3], f32, tag="hx")
                    nc.vector.tensor_tensor(
                        out=hx[:, :, :], in0=v[0:nq, 1 : w + 2, :],
                        in1=v[0:nq, 0 : w + 1, :], op=Alu.subtract,
                    )
                    hd = wk.tile([nq, w + 1, 3], f32, tag="hd")
                    nc.vector.tensor_tensor(
                        out=hd[:, :, :], in0=vd[:, 1 : w + 2, :],
                        in1=vd[:, 0 : w + 1, :], op=Alu.subtract,
                    )
                    vy = wk.tile([nq, w + 2, 3], f32, tag="vy")
                    nc.vector.tensor_tensor(
                        out=vy[:, :, :], in0=vd[:, :, :], in1=v[0:nq, :, :],
                        op=Alu.subtract,
                    )
                    dd = wk.tile([nq, w + 1, 3], f32, tag="dd")
                    nc.gpsimd.tensor_tensor(
                        out=dd[:, :, :], in0=hx[:, :, :], in1=vy[:, 1 : w + 2, :],
                        op=Alu.add,
                    )

                    # cross products: C1 = hx x vy(c+1),  C2 = hd x d
                    m1 = wk.tile([nq, w + 1, 3], f32, tag="m1")
                    m2 = wk.tile([nq, w + 1, 3], f32, tag="m2")
                    c1 = wk.tile([nq, w + 1, 3], f32, tag="c1")
                    c2 = wk.tile([nq, w + 1, 3], f32, tag="c2")
                    for k in range(3):
                        u, x = (k + 1) % 3, (k + 2) % 3
                        nc.vector.tensor_tensor(
                            out=m1[:, :, k : k + 1],
                            in0=hx[:, :, u : u + 1],
                            in1=vy[:, 1 : w + 2, x : x + 1], op=Alu.mult,
                        )
                        nc.vector.tensor_tensor(
                            out=m2[:, :, k : k + 1],
                            in0=hx[:, :, x : x + 1],
                            in1=vy[:, 1 : w + 2, u : u + 1], op=Alu.mult,
                        )
                    nc.vector.tensor_tensor(
                        out=c1[:, :, :], in0=m1[:, :, :], in1=m2[:, :, :],
                        op=Alu.subtract,
                    )
                    for k in range(3):
                        u, x = (k + 1) % 3, (k + 2) % 3
                        nc.vector.tensor_tensor(
                            out=m1[:, :, k : k + 1],
                            in0=hd[:, :, u : u + 1],
                            in1=dd[:, :, x : x + 1], op=Alu.mult,
                        )
                        nc.vector.tensor_tensor(
                            out=m2[:, :, k : k + 1],
                            in0=hd[:, :, x : x + 1],
                            in1=dd[:, :, u : u + 1], op=Alu.mult,
                        )
                    nc.vector.tensor_tensor(
                        out=c2[:, :, :], in0=m1[:, :, :], in1=m2[:, :, :],
                        op=Alu.subtract,
                    )

                    # T = C1 + C2; P/Q split of the stencil by quad row parity
                    t = wk.tile([nq, w + 1, 3], f32, tag="t")
                    nc.gpsimd.tensor_tensor(
                        out=t[:, :, :], in0=c1[:, :, :], in1=c2[:, :, :],
                        op=Alu.add,
                    )
                    p = wk.tile([nq, w, 3], f32, tag="p")
                    nc.vector.tensor_tensor(
                        out=p[:, :, :], in0=t[:, 1 : w + 1, :], in1=c1[:, 0:w, :],
                        op=Alu.add,
                    )
                    q = wk.tile([nq, w, 3], f32, tag="q")
                    nc.gpsimd.tensor_tensor(
                        out=q[:, :, :], in0=t[:, 0:w, :], in1=c2[:, 1 : w + 1, :],
                        op=Alu.add,
                    )

                    # S = shift_down(P) + Q  (shift on the tensor engine)
                    s = wk.tile([ns, w, 3], f32, tag="s")
                    for j0, pw in _col_chunks(w, PSUM_COLS):
                        ps = psp.tile([128, pw, 3], f32, tag="ps")
                        nc.tensor.matmul(
                            out=ps[:, :, :],
                            lhsT=tid[0:nq, 1:129],
                            rhs=p[:, j0 : j0 + pw, :],
                            start=True, stop=True,
                        )
                        nc.vector.tensor_tensor(
                            out=s[:, j0 : j0 + pw, :],
                            in0=ps[0:ns, :, :],
                            in1=q[0:ns, j0 : j0 + pw, :], op=Alu.add,
                        )

                    # normalization
                    sq = wk.tile([ns, w, 3], f32, tag="sq")
                    nc.scalar.activation(
                        out=sq[:, :, :], in_=s[:, :, :], func=Act.Square
                    )
                    nsq = wk.tile([ns, w, 1], f32, tag="nsq")
                    nc.vector.tensor_tensor(
                        out=nsq[:, :, :], in0=sq[:, :, 0:1], in1=sq[:, :, 1:2],
                        op=Alu.add,
                    )
                    nc.vector.tensor_tensor(
                        out=nsq[:, :, :], in0=nsq[:, :, :], in1=sq[:, :, 2:3],
                        op=Alu.add,
                    )
                    rn = wk.tile([ns, w, 1], f32, tag="rn")
                    nc.scalar.activation(
                        out=rn[:, :, :], in_=nsq[:, :, :], func=Act.Sqrt,
                        bias=eps_tile[:ns, :],
                    )
                    nc.vector.reciprocal(out=rn[:, :, :], in_=rn[:, :, :])
                    o = io.tile([ns, w, 3], f32, tag="o")
                    for k in range(3):
                        nc.vector.tensor_tensor(
                            out=o[:, :, k : k + 1],
                            in0=s[:, :, k : k + 1], in1=rn[:, :, :],
                            op=Alu.mult,
                        )
                    nc.sync.dma_start(
                        out=oband[r0 : r0 + ns, c0 : c0 + w, :], in_=o[:, :, :]
                    )


_PROGRAM_CACHE: dict = {}


def _get_program(grid: int, n_cores: int, repeats: int = 1):
    key = (grid, n_cores, repeats)
    if key not in _PROGRAM_CACHE:
        _PROGRAM_CACHE[key] = _build_program(grid, n_cores, repeats)
    return _PROGRAM_CACHE[key]


def _run_stencil_on_device(vertices: np.ndarray, grid: int, n_cores: int,
                           trace: bool = False, repeats: int = 1):
    from concourse.bass_utils import run_bass_kernel_spmd

    base, out_rows, in_rows = _band_layout(grid, n_cores)
    V = np.ascontiguousarray(vertices.reshape(grid, grid, 3).astype(np.float32))
    VP = np.pad(V, ((1, 1), (1, 1), (0, 0)), mode="edge")

    in_maps = [
        {"vband": np.ascontiguousarray(VP[base * k : base * k + in_rows])}
        for k in range(n_cores)
    ]
    nc = _get_program(grid, n_cores, repeats)
    kres = run_bass_kernel_spmd(nc, in_maps, list(range(n_cores)), trace=trace)

    out = np.empty((grid, grid, 3), dtype=np.float32)
    for k in range(n_cores):
        ob = kres.results[k]["oband"]
        take = out_rows - 1 if k < n_cores - 1 else out_rows
        out[base * k : base * k + take] = ob[:take]
    return out.reshape(grid * grid, 3), kres


def kernel(vertices: np.ndarray, faces: np.ndarray) -> np.ndarray:
    vertices = np.asarray(vertices, dtype=np.float32)
    faces = np.asarray(faces)
    grid = int(round(np.sqrt(vertices.shape[0])))
    if (
        grid * grid == vertices.shape[0]
        and (grid - 1) % N_CORES == 0
        and _is_structured(faces, grid)
    ):
        out, _ = _run_stencil_on_device(vertices, grid, N_CORES)
        return out
    print("kernel: faces are not the structured triangulation; host fallback",
          file=sys.stderr)
    return _host_fallback(vertices, faces)
